# revision 3
# baseline (speedup 1.0000x reference)
"""AttentionFreeTransformer distributed Bass kernel for one TRN2 chip (8 NeuronCores).

Math (exp_pos_bias == exp(0) == 1 exactly, so W_bias is unused and the bias
einsum collapses to a sum over j):

    Q = q @ Wq ; K = k @ Wk ; V = v @ Wv            # [B,T,DH]
    m[j,d]   = max_b K[b,j,d]
    w        = exp(K - m)
    num[b,d] = sum_j w[b,j,d] * V[b,j,d]            (independent of the query i)
    den[b,d] = sum_j w[b,j,d]
    out      = (sigmoid(Q) * num/den) @ Wo          # [B,T,DM]

Sharding: sequence-parallel over T (T/8 = 256 rows per core, all 4 batches).
m = max over b is core-local; only the 8 KB num/den partial sums couple the
cores.  TWO SPMD launches with a host-side 8 KB reduction in between -- any
in-kernel cross-core sync (ncfw collective or raw remote_dma_broadcast) eats
the ~60us axon per-core launch skew (measured 130-200us end-to-end), so two
independent skew-immune launches win.

  L1: in-projections (bf16), m/exp/partials, sigmoid(Q) -> sig + 16-col partials.
  host: sum partials over cores, ratio = num/den, yt = sigmoid * ratio (tiny).
  L2: out-projection  out = yt^T @ Wo  -> bf16 output tiles.

Schedule notes (each item trace-verified):
  - One DMA descriptor per partition line at ~26GB/s/queue x 16 queues: all
    inputs ride ONE HWDGE ring (sync) in strict PE-consumption order with
    4KB+ lines; outputs ride the scalar ring.
  - Projection order K, Q, V.  Whichever projection is LAST drags its serial
    post-processing into the tail, so V's arrival is row-half-major: pass A
    (rows 0-511) finishes early and its fused multiply-reduce partial sums
    hide under pass B's matmuls -- only ~4 matmuls + 4 multiply-reduces +
    the 8KB partials store trail the final input transfer.
  - kc-OUTER accumulation for Q keeps all PSUM regions filling in parallel
    with the q stream; the sigmoid chain hides under the V matmuls.
  - psK PSUM slots are released via one bulk PSUM->SBUF copy (psQ/psV tiles
    recycle them; the slow max/sub chain must not hold them).
  - Junk matmuls bridge until the first k chunk lands: idle PE gaps drop the
    HAM clock gate and matmuls re-ramp 216->600ns.
  - L2: rc-major yt layout (one chunk gates only the first rc pair),
    full-width [P,1024] PSUM drains alternating vector/scalar (final rc
    split across both), partition-major DRAM output so rc-pair stores have
    4KB partition lines.

History: 72.0us baseline -> 70.1 (hide V partial-sums) -> 68.4 (row-half V).
L1 41.3us (preamble 7.2 + 7.5MB stream at ~306GB/s + ~3 tail), L2 27.2us.
"""

import numpy as np
import ml_dtypes

import concourse.bacc as bacc_mod
import concourse.mybir as mybir
import concourse.tile as tile
from concourse.bass_utils import run_bass_kernel_spmd

B, T, DM, DH = 4, 2048, 1024, 256
NCORES = 8
TLOC = T // NCORES          # 256 sequence rows per core
R = B * TLOC                # 1024 (b, j) rows per core
P = 128
KC = DM // P                # 8 contraction chunks for the in-projections
MC = DH // P                # 2 dh chunks
RC = R // P                 # 8 row chunks
NT = DM // 512              # 2 out-proj free tiles
BF16 = mybir.dt.bfloat16
F32 = mybir.dt.float32

_CACHE: dict = {}


def build_front():
    AF = mybir.ActivationFunctionType
    ALU = mybir.AluOpType
    nc = bacc_mod.Bacc(num_devices=NCORES)
    qT = nc.declare_dram_parameter("qT", [P, KC * R], BF16, isOutput=False)
    kT = nc.declare_dram_parameter("kT", [P, KC * R], BF16, isOutput=False)
    vT = nc.declare_dram_parameter("vT", [P, 2, KC, 512], BF16, isOutput=False)
    wq = nc.declare_dram_parameter("wq", [P, KC, DH], BF16, isOutput=False)
    wk = nc.declare_dram_parameter("wk", [P, KC, DH], BF16, isOutput=False)
    wv = nc.declare_dram_parameter("wv", [P, KC, DH], BF16, isOutput=False)
    sig_out = nc.declare_dram_parameter("sig", [P, MC * R], BF16, isOutput=True)
    part_out = nc.declare_dram_parameter("part", [P, 16], F32, isOutput=True)

    with tile.TileContext(nc) as tc:
        with (
            tc.tile_pool(name="big", bufs=1) as big,
            tc.tile_pool(name="small", bufs=4) as small,
            tc.tile_pool(name="psum", bufs=4, space="PSUM") as psum,
        ):
            wv_sb = big.tile([P, KC, DH], BF16, tag="wv_sb")
            wk_sb = big.tile([P, KC, DH], BF16, tag="wk_sb")
            wq_sb = big.tile([P, KC, DH], BF16, tag="wq_sb")
            v_sb = big.tile([P, 2, KC, 512], BF16, tag="v_sb")
            k_sb = big.tile([P, KC, R], BF16, tag="k_sb")
            q_sb = big.tile([P, KC, R], BF16, tag="q_sb")
            m_sb = big.tile([P, MC, TLOC], F32, tag="m_sb")
            kcopy = big.tile([P, MC, R], F32, tag="kcopy")
            wpre = big.tile([P, MC, R], F32, tag="wpre")
            wexp = big.tile([P, MC, R], F32, tag="wexp")
            sig = big.tile([P, MC, R], BF16, tag="sig")
            partials = big.tile([P, 16], F32, tag="partials")

            # PE warm-up: junk matmuls lift the HAM clock gate to 2.4GHz and
            # HOLD it until the first real matmuls (idle gaps drop it again)
            wm = big.tile([P, 512], BF16, tag="wm")
            nc.gpsimd.memset(wm[:], 0.0)
            ps_warm = psum.tile([P, R], F32, tag="mm", name="ps_warm")
            for i in range(14):
                nc.tensor.matmul(ps_warm[:, 0:512], wm[:, 0:P], wm[:],
                                 start=True, stop=True)

            # ALL inputs on ONE ring (sync), strictly in PE-consumption
            # order: the 16 SDMA queues round-robin between the two HWDGE
            # rings, so anything on the other ring delays the first-needed
            # transfer 1:1. Scalar ring only carries late outputs.
            CHK = KC * R // 8
            HLF = KC * R // 2
            nc.sync.dma_start(wk_sb[:], wk[:])
            nc.sync.dma_start(k_sb[:, 0:2, :], kT[:, 0:2 * CHK])
            nc.sync.dma_start(k_sb[:, 2:5, :], kT[:, 2 * CHK:5 * CHK])
            nc.sync.dma_start(k_sb[:, 5:8, :], kT[:, 5 * CHK:])
            nc.sync.dma_start(wq_sb[:], wq[:])
            nc.sync.dma_start(q_sb[:, 0:4, :], qT[:, 0:HLF])
            nc.sync.dma_start(q_sb[:, 4:8, :], qT[:, HLF:])
            nc.sync.dma_start(wv_sb[:], wv[:])
            nc.sync.dma_start(v_sb[:, 0, :, :], vT[:, 0, :, :])
            nc.sync.dma_start(v_sb[:, 1, 0:4, :], vT[:, 1, 0:4, :])
            nc.sync.dma_start(v_sb[:, 1, 4:8, :], vT[:, 1, 4:8, :])

            def in_proj(x_sb, w_sb, ps_tiles):
                for mc in range(MC):
                    for kc in range(KC):
                        for rt in range(2):
                            nc.tensor.matmul(
                                ps_tiles[mc][:, rt * 512:(rt + 1) * 512],
                                w_sb[:, kc, mc * P:(mc + 1) * P],
                                x_sb[:, kc, rt * 512:(rt + 1) * 512],
                                start=(kc == 0),
                                stop=(kc == KC - 1),
                            )

            # K projection -> m = max_b K -> w = exp(K - m) (+ den partials)
            psK = [psum.tile([P, R], F32, tag="mm", name=f"psK{mc}") for mc in range(MC)]
            in_proj(k_sb, wk_sb, psK)
            for mc in range(MC):
                # one bulk PSUM->SBUF copy releases psK's PSUM slot fast --
                # the psQ tiles recycle these slots, so the Q matmuls must
                # not wait for the whole max/sub chain
                nc.vector.tensor_copy(kcopy[:, mc, :], psK[mc][:])
            for mc in range(MC):
                nc.vector.tensor_copy(m_sb[:, mc, :], kcopy[:, mc, 0:TLOC])
                for b in range(1, B):
                    nc.vector.tensor_max(m_sb[:, mc, :], m_sb[:, mc, :],
                                         kcopy[:, mc, b * TLOC:(b + 1) * TLOC])
                for b in range(B):
                    sl = slice(b * TLOC, (b + 1) * TLOC)
                    nc.vector.tensor_sub(wpre[:, mc, sl], kcopy[:, mc, sl], m_sb[:, mc, :])
                    # exp with fused free-dim sum -> den partial
                    nc.scalar.activation(
                        wexp[:, mc, sl], wpre[:, mc, sl], AF.Exp,
                        accum_out=partials[:, 8 + mc * 4 + b: 9 + mc * 4 + b],
                    )

            # Q projection + sigmoid, SECOND (V goes last: its tail work
            # after the stream ends is ~1.7us of partial-sums, vs ~2.8us of
            # serial sigmoids). kc-OUTER so all four 512-wide PSUM regions
            # accumulate in parallel across the q chunk stream.
            psQ = [psum.tile([P, R], F32, tag="mm", name=f"psQ{mc}") for mc in range(MC)]
            for kc in range(KC):
                for mc in range(MC):
                    for rt in range(2):
                        nc.tensor.matmul(
                            psQ[mc][:, rt * 512:(rt + 1) * 512],
                            wq_sb[:, kc, mc * P:(mc + 1) * P],
                            q_sb[:, kc, rt * 512:(rt + 1) * 512],
                            start=(kc == 0),
                            stop=(kc == KC - 1),
                        )
            for mc in range(MC):
                for rt in range(2):
                    sl = slice(rt * 512, (rt + 1) * 512)
                    nc.scalar.activation(sig[:, mc, sl], psQ[mc][:, sl], AF.Sigmoid)
                    # sigmoid stores ride the scalar ring mid-kernel; the
                    # sync ring still holds queued v descriptors
                    nc.scalar.dma_start(sig_out[:, mc * R + rt * 512: mc * R + (rt + 1) * 512],
                                  sig[:, mc, sl])

            # V projection LAST, row-half pipelined: v arrives rt0-rows
            # first, pass A computes the rt0 PSUM regions and its fused
            # multiply-reduces hide under pass B's matmuls. Only ~4 matmuls,
            # 4 multiply-reduces and the 8KB store trail the final transfer.
            psV = [psum.tile([P, R], F32, tag="mm", name=f"psV{mc}") for mc in range(MC)]
            for rt in range(2):
                for kc in range(KC):
                    for mc in range(MC):
                        nc.tensor.matmul(
                            psV[mc][:, rt * 512:(rt + 1) * 512],
                            wv_sb[:, kc, mc * P:(mc + 1) * P],
                            v_sb[:, rt, kc, :],
                            start=(kc == 0),
                            stop=(kc == KC - 1),
                        )
                for mc in range(MC):
                    for b in (2 * rt, 2 * rt + 1):
                        sl = slice(b * TLOC, (b + 1) * TLOC)
                        scr = small.tile([P, TLOC], F32, tag="scr", name=f"scr{mc}_{b}")
                        # fused multiply-reduce (tensor_tensor_reduce crashes
                        # this runtime; scalar_tensor_tensor works)
                        nc.vector.scalar_tensor_tensor(
                            scr[:], wexp[:, mc, sl], 1.0, psV[mc][:, sl],
                            ALU.mult, ALU.mult,
                            accum_out=partials[:, mc * 4 + b: mc * 4 + b + 1],
                        )
            nc.scalar.dma_start(part_out[:], partials[:])

    nc._bir_kernel_barrier_sem_replica_groups = []
    nc.compile()
    return nc


def build_back():
    AF = mybir.ActivationFunctionType
    nc = bacc_mod.Bacc(num_devices=NCORES)
    # yt is rc-major on the host: [P, RC, MC*128] so one [P, 2, MC*P]
    # chunk gates only the first two rc's out-proj, not the whole tensor
    yt_in = nc.declare_dram_parameter("yt", [P, RC, MC * P], BF16, isOutput=False)
    wo = nc.declare_dram_parameter("wo", [P, MC, DM], BF16, isOutput=False)
    # partition-major output: per-partition lines of rc-pairs are 4KB
    # contiguous, halving descriptor count per byte (host transposes back)
    out = nc.declare_dram_parameter("out", [P, RC, DM], BF16, isOutput=True)

    with tile.TileContext(nc) as tc:
        with (
            tc.tile_pool(name="big", bufs=1) as big,
            tc.tile_pool(name="osb", bufs=3) as osb,
            tc.tile_pool(name="psum", bufs=4, space="PSUM") as psum,
        ):
            yt = big.tile([P, RC, MC * P], BF16, tag="yt")
            wo_sb = big.tile([P, MC, DM], BF16, tag="wo_sb")
            wm = big.tile([P, 512], BF16, tag="wm")
            nc.gpsimd.memset(wm[:], 0.0)
            ps_warm = psum.tile([P, DM], F32, tag="mm", name="ps_warm")
            for i in range(14):
                nc.tensor.matmul(ps_warm[:, 0:512], wm[:, 0:P], wm[:],
                                 start=True, stop=True)

            # all inputs on the sync ring in consumption order; the first
            # matmul is gated on just yt[0:2]+wo[0]. Outputs go on scalar.
            nc.sync.dma_start(yt[:, 0:2, :], yt_in[:, 0:2, :])
            nc.sync.dma_start(wo_sb[:, 0, :], wo[:, 0, :])
            nc.sync.dma_start(wo_sb[:, 1, :], wo[:, 1, :])
            nc.sync.dma_start(yt[:, 2:8, :], yt_in[:, 2:8, :])

            for rp in range(RC // 2):
                o_sb = osb.tile([P, 2, DM], BF16, tag="o_sb", name=f"o_sb{rp}")
                for h in range(2):
                    rc = 2 * rp + h
                    psO = psum.tile([P, DM], F32, tag="mm", name=f"psO{rc}")
                    for mc in range(MC):
                        for nt in range(NT):
                            nc.tensor.matmul(
                                psO[:, nt * 512:(nt + 1) * 512],
                                yt[:, rc, mc * P:(mc + 1) * P],
                                wo_sb[:, mc, nt * 512:(nt + 1) * 512],
                                start=(mc == 0),
                                stop=(mc == MC - 1),
                            )
                    # full-width drain, engines alternating per rc; the
                    # final rc drains as two halves on both engines at once
                    if rc == RC - 1:
                        nc.vector.tensor_copy(o_sb[:, h, 0:512], psO[:, 0:512])
                        nc.scalar.activation(o_sb[:, h, 512:], psO[:, 512:], AF.Copy)
                    elif rc % 2 == 0:
                        nc.vector.tensor_copy(o_sb[:, h, :], psO[:])
                    else:
                        nc.scalar.activation(o_sb[:, h, :], psO[:], AF.Copy)
                # one 4KB-per-partition-line store per rc pair on the sync
                # ring, whose input descriptors have drained by then (the
                # scalar queue is busy with the COPY drains)
                nc.sync.dma_start(out[:, 2 * rp:2 * rp + 2, :], o_sb[:])

    nc._bir_kernel_barrier_sem_replica_groups = []
    nc.compile()
    return nc


def get_ncs():
    if "ncs" not in _CACHE:
        _CACHE["ncs"] = (build_front(), build_back())
    return _CACHE["ncs"]


def make_front_maps(q, k, v, Wq, Wk, Wv):
    bf = ml_dtypes.bfloat16
    wq_h = np.ascontiguousarray(np.asarray(Wq, np.float32).reshape(KC, P, DH).transpose(1, 0, 2)).astype(bf)
    wk_h = np.ascontiguousarray(np.asarray(Wk, np.float32).reshape(KC, P, DH).transpose(1, 0, 2)).astype(bf)
    wv_h = np.ascontiguousarray(np.asarray(Wv, np.float32).reshape(KC, P, DH).transpose(1, 0, 2)).astype(bf)
    maps = []
    for c in range(NCORES):
        sl = slice(c * TLOC, (c + 1) * TLOC)

        def xt(x):
            # [B, TLOC, DM] -> [DM, B, TLOC] -> [P, KC*R]: 16KB contiguous
            # per SBUF partition
            a = x[:, sl, :].transpose(2, 0, 1).reshape(KC, P, R)
            return np.ascontiguousarray(a.transpose(1, 0, 2)).reshape(P, KC * R).astype(bf)
        vt = np.ascontiguousarray(
            xt(v).reshape(P, KC, 2, 512).transpose(0, 2, 1, 3))
        maps.append({"qT": xt(q), "kT": xt(k), "vT": vt,
                     "wq": wq_h, "wk": wk_h, "wv": wv_h})
    return maps


def make_back_maps(front_results, Wo):
    bf = ml_dtypes.bfloat16
    # host AllReduce of the 8KB partials: cols [0:8]=num, [8:16]=den (mc*4+b)
    parts = np.zeros((P, 16), np.float64)
    for c in range(NCORES):
        parts += np.asarray(front_results[c]["part"], np.float64)
    ratio = (parts[:, 0:8] / parts[:, 8:16]).astype(np.float32)  # [P, mc*4+b]
    wo_h = np.ascontiguousarray(
        np.asarray(Wo, np.float32).reshape(MC, P, DM).transpose(1, 0, 2)).astype(bf)
    rat = ratio.reshape(P, MC, B, 1)
    maps = []
    for c in range(NCORES):
        sig = np.asarray(front_results[c]["sig"]).astype(np.float32)
        sig = sig.reshape(P, MC, B, TLOC)
        yt = (sig * rat).astype(bf).reshape(P, MC, R)
        # rc-major for chunked device DMA: [P, RC, MC*128]
        yt = np.ascontiguousarray(
            yt.reshape(P, MC, RC, P).transpose(0, 2, 1, 3)).reshape(P, RC, MC * P)
        maps.append({"yt": yt, "wo": wo_h})
    return maps


def assemble(back_results):
    outp = np.empty((B, T, DM), np.float32)
    for c in range(NCORES):
        sl = slice(c * TLOC, (c + 1) * TLOC)
        # device wrote [P, RC, DM] partition-major; row index is rc*128+p
        o = np.asarray(back_results[c]["out"]).astype(np.float32)
        outp[:, sl, :] = o.transpose(1, 0, 2).reshape(B, TLOC, DM)
    return outp


def kernel(q, k, v, Wq, Wk, Wv, Wo, W_bias=None, **_unused):
    q = np.asarray(q, np.float32)
    k = np.asarray(k, np.float32)
    v = np.asarray(v, np.float32)
    nc1, nc2 = get_ncs()
    fmaps = make_front_maps(q, k, v, Wq, Wk, Wv)
    r1 = run_bass_kernel_spmd(nc1, fmaps, list(range(NCORES)))
    bmaps = make_back_maps(r1.results, Wo)
    r2 = run_bass_kernel_spmd(nc2, bmaps, list(range(NCORES)))
    return assemble(r2.results)

